# revision 16
# baseline (speedup 1.0000x reference)
"""Trainium2 Bass kernel for nn_Attention_8589935148.

Attention with softmax over the HEAD axis (dim=1), returning (out, p_attn).

Math notes (B=2, H=16, S=2048, D=64):
  scores s[b,h,q,k] = (q . k) / 8;  mask[b,1,q,k] broadcasts over h, so a
  masked (b,q,k) position is masked for ALL 16 heads -> softmax over h of 16
  equal NEG values = uniform 1/16.  Exact reformulation used here:
      E   = exp(s)                      (raw, unmasked scores)
      F   = (E - 1) * mask              (0 at masked positions)
      Z   = (sum_h E - 16) * mask + 16  (= sum_h exp(s) unmasked; 16 masked)
      p   = (F + 1) / Z                 (= softmax_h unmasked; 1/16 masked)
      out = p @ v
  This needs no select/branch ops: two scalar_tensor_tensor instructions per
  head-tile plus one reciprocal per (q,k) plane.

Sharding: batch x query-block. Core c handles batch c//4, query rows
[(c%4)*512, (c%4+1)*512). All 16 heads stay on-core, so the softmax over
heads is purely local — zero collectives.
"""

import sys

sys.path.insert(0, "/opt/trn_rl_repo")

import numpy as np
import ml_dtypes

import concourse.bass as bass
import concourse.tile as tile
from concourse import bacc, mybir
from concourse.bass_utils import run_bass_kernel_spmd
from concourse.masks import make_identity

F32 = mybir.dt.float32
F32R = mybir.dt.float32r
BF16 = mybir.dt.bfloat16
BF16_NP = np.dtype(ml_dtypes.bfloat16)

B, H, S, D = 2, 16, 2048, 64
NCORES = 8
QBLK = S // 4          # 512 query rows per core
QT = 128               # query tile (partition dim)
NQT = QBLK // QT       # 4 q-tiles per core
KC = 1024              # k chunk width processed per softmax round
NKC = S // KC          # 2
KB = 128               # k sub-block for the PV matmul
NKB = KC // KB         # 8
SCALE = 0.125          # 1/sqrt(64)

ALU = mybir.AluOpType
ACTF = mybir.ActivationFunctionType

# feature flags (perf/bisect knobs)
USE_F32R = True         # float32r QK matmul (full PE rate) vs bf16
QK_DT = F32R if USE_F32R else BF16
XBAR_PT = True          # DMA-xbar transpose of bf16 P (else PE transpose)
GP_UPCAST = True        # GpSimd helps with the bf16->f32 p_attn upcast


def build_nc():
    nc = bacc.Bacc("TRN2", target_bir_lowering=False, debug=False,
                   num_devices=NCORES)

    # Inputs (per-core shards, pre-laid-out on host)
    qT = nc.dram_tensor("qT", [H // 2, 128, QBLK], QK_DT, kind="ExternalInput").ap()
    kT = nc.dram_tensor("kT", [H // 2, 128, S], QK_DT, kind="ExternalInput").ap()
    vp = nc.dram_tensor("vp", [S // KB, KB, H, D], BF16, kind="ExternalInput").ap()
    mk = nc.dram_tensor("mk", [NQT, QT, S], BF16, kind="ExternalInput").ap()
    # Outputs
    p_out = nc.dram_tensor("p_out", [H, QBLK, S], F32, kind="ExternalOutput").ap()
    o_out = nc.dram_tensor("o_out", [H, QBLK, D], F32, kind="ExternalOutput").ap()

    with tile.TileContext(nc) as tc:
        with (
            tc.tile_pool(name="const", bufs=1) as const_pool,
            tc.tile_pool(name="kTp", bufs=H // 2) as kT_pool,
            tc.tile_pool(name="qTp", bufs=H // 2) as qT_pool,
            tc.tile_pool(name="vpp", bufs=S // KB) as v_pool,
            tc.tile_pool(name="mkp", bufs=2) as m_pool,
            tc.tile_pool(name="Ep", bufs=16) as E_pool,
            tc.tile_pool(name="ztp", bufs=4) as zt_pool,
            tc.tile_pool(name="zp", bufs=1) as z_pool,
            tc.tile_pool(name="rp", bufs=2) as r_pool,
            tc.tile_pool(name="pbp", bufs=2) as pb_pool,
            tc.tile_pool(name="pp", bufs=2) as p_pool,
            tc.tile_pool(name="ptp", bufs=2) as pt_pool,
            tc.tile_pool(name="op", bufs=1) as o_pool,
            tc.tile_pool(name="spsum", bufs=3, space="PSUM") as s_psum,
            tc.tile_pool(name="ptpsum", bufs=1, space="PSUM") as pt_psum,
            tc.tile_pool(name="opsum", bufs=1, space="PSUM") as o_psum,
        ):
            ident_bf = const_pool.tile([128, 128], BF16)
            make_identity(nc, ident_bf[:])

            # ---- resident loads --------------------------------------
            kT_sb = []
            for i in range(H // 2):
                t = kT_pool.tile([128, S], QK_DT, tag="kT")
                nc.sync.dma_start(out=t, in_=kT[i])
                kT_sb.append(t)
            qT_sb = []
            for i in range(H // 2):
                t = qT_pool.tile([128, QBLK], QK_DT, tag="qT")
                nc.sync.dma_start(out=t, in_=qT[i])
                qT_sb.append(t)
            v_sb = []
            for i in range(S // KB):
                t = v_pool.tile([KB, H, D], BF16, tag="vp")
                nc.sync.dma_start(out=t, in_=vp[i])
                v_sb.append(t)
            # ---- main loop -------------------------------------------
            for qt in range(NQT):
                o_ps = o_psum.tile([QT, H, D], F32, tag="ops")
                m_t = m_pool.tile([QT, S], BF16, tag="mk")
                nc.sync.dma_start(out=m_t, in_=mk[qt])
                for kc in range(NKC):
                    # scores + exp for all 16 heads
                    Es = []
                    for h in range(H):
                        pair, half = h // 2, h % 2
                        lo = 64 * half
                        s_ps = s_psum.tile([QT, KC], F32, tag="sps")
                        lhsT = qT_sb[pair][lo:lo + 64, qt * QT:(qt + 1) * QT]
                        for j in range(KC // 512):
                            rhs = kT_sb[pair][lo:lo + 64,
                                              kc * KC + j * 512: kc * KC + (j + 1) * 512]
                            nc.tensor.matmul(
                                s_ps[:, j * 512:(j + 1) * 512],
                                lhsT=lhsT, rhs=rhs,
                                start=True, stop=True,
                            )
                        E_h = E_pool.tile([QT, KC], BF16, tag="E")
                        nc.scalar.activation(E_h, s_ps, ACTF.Exp, scale=SCALE)
                        Es.append(E_h)

                    # Z = sum_h E  (pairwise tree, bf16, fp32 root)
                    Z = z_pool.tile([QT, KC], F32, tag="Z")
                    for sub in range(KC // 512):
                        sl = slice(sub * 512, (sub + 1) * 512)
                        T = [zt_pool.tile([QT, 512], BF16, tag="zt",
                                          name=f"zt{qt}_{kc}_{sub}_{j}")
                             for j in range(4)]
                        for j in range(4):
                            nc.vector.tensor_add(T[j], Es[4 * j][:, sl],
                                                 Es[4 * j + 1][:, sl])
                            nc.vector.tensor_add(T[j], T[j],
                                                 Es[4 * j + 2][:, sl])
                            nc.vector.tensor_add(T[j], T[j],
                                                 Es[4 * j + 3][:, sl])
                        nc.vector.tensor_add(T[0], T[0], T[1])
                        nc.vector.tensor_add(T[2], T[2], T[3])
                        nc.vector.tensor_add(Z[:, sl], T[0], T[2])

                    # Zc = (Z - 16) * mask + 16 ;  R = 1/Zc
                    mt = m_t[:, kc * KC:(kc + 1) * KC]
                    nc.vector.scalar_tensor_tensor(
                        out=Z, in0=Z, scalar=-16.0, in1=mt,
                        op0=ALU.add, op1=ALU.mult)
                    nc.vector.tensor_scalar_add(Z, Z, 16.0)
                    R = r_pool.tile([QT, KC], F32, tag="R")
                    nc.vector.reciprocal_approx_fast(R, Z)
                    Hm = r_pool.tile([QT, KC], BF16, tag="Hm")
                    nc.vector.tensor_mul(Hm, R, mt)          # H = R*mask
                    Cm = r_pool.tile([QT, KC], BF16, tag="Cm")
                    nc.vector.scalar_tensor_tensor(
                        out=Cm, in0=Hm, scalar=-1.0, in1=R,
                        op0=ALU.mult, op1=ALU.add)           # C = R - H

                    # normalize + transpose + PV per head
                    for h in range(H):
                        E_h = Es[h]
                        # P = E*H + C  (bf16 chain, 2x mode)
                        Tm = pb_pool.tile([QT, KC], BF16, tag="Tm")
                        nc.vector.tensor_mul(Tm, E_h, Hm)
                        Pb = pb_pool.tile([QT, KC], BF16, tag="Pb")
                        nc.vector.tensor_add(Pb, Tm, Cm)
                        # fp32 upcast for the p_attn payload
                        P = p_pool.tile([QT, KC], F32, tag="P")
                        if GP_UPCAST and h % 2 == 0:
                            nc.gpsimd.tensor_copy(out=P, in_=Pb)
                        else:
                            nc.scalar.copy(P, Pb)
                        dma_eng = nc.sync if h % 2 == 0 else nc.scalar
                        dma_eng.dma_start(
                            out=p_out[h, qt * QT:(qt + 1) * QT,
                                      kc * KC:(kc + 1) * KC],
                            in_=P)
                        # PT = P^T
                        PT = pt_pool.tile([QT, KC], BF16, tag="PT")
                        pb3 = Pb.rearrange("q (kb ko) -> q kb ko", ko=KB)
                        pt3 = PT.rearrange("k (kb qo) -> k kb qo", qo=QT)
                        for kb in range(NKB):
                            teng = nc.sync if (h + kb) % 2 == 0 else nc.scalar
                            teng.dma_start(out=pt3[:, kb, :],
                                           in_=pb3[:, kb, :],
                                           transpose=True)
                        # out[q,d] += P^T.T @ V
                        for kb in range(NKB):
                            kbg = kc * NKB + kb
                            nc.tensor.matmul(
                                o_ps[:, h, :],
                                lhsT=PT[:, kb * KB:(kb + 1) * KB],
                                rhs=v_sb[kbg][:, h, :],
                                start=(kc == 0 and kb == 0 and h % 8 == 0),
                                stop=(kc == NKC - 1 and kb == NKB - 1
                                      and h % 8 == 7),
                            )

                # evict attention output for this q-tile
                O = o_pool.tile([QT, H, D], F32, tag="O")
                nc.scalar.copy(O, o_ps)
                for h in range(H):
                    nc.sync.dma_start(
                        out=o_out[h, qt * QT:(qt + 1) * QT, :], in_=O[:, h, :])

    nc.compile()
    return nc


_NC_CACHE = None


def _get_nc():
    global _NC_CACHE
    if _NC_CACHE is None:
        _NC_CACHE = build_nc()
    return _NC_CACHE


def _prep_core_inputs(query, key, value, mask):
    """Host-side shard + relayout. Returns list of 8 in_maps."""
    q = np.asarray(query, np.float32)
    k = np.asarray(key, np.float32)
    v = np.asarray(value, np.float32)
    m = np.asarray(mask)

    in_maps = []
    for b in range(B):
        # K^T packed: [16,2048,64] -> [16,64,2048] -> [8,128,2048]
        kTb = np.ascontiguousarray(k[b].transpose(0, 2, 1)).reshape(H // 2, 128, S)
        # V packed: [16,2048,64] -> [2048,16,64] -> [16,128,16,64] bf16
        vpb = np.ascontiguousarray(v[b].transpose(1, 0, 2)).reshape(
            S // KB, KB, H, D).astype(BF16_NP)
        for qi in range(NCORES // B):
            q0 = qi * QBLK
            qTb = np.ascontiguousarray(
                q[b, :, q0:q0 + QBLK, :].transpose(0, 2, 1)).reshape(
                H // 2, 128, QBLK)
            mkb = m[b, 0, q0:q0 + QBLK, :].astype(BF16_NP).reshape(NQT, QT, S)
            in_maps.append({"qT": qTb, "kT": kTb, "vp": vpb, "mk": mkb})
    # core order: b-major then q-block -> core c = b*4 + qi
    return in_maps


def run_on_cores(query, key, value, mask, trace=False, **kw):
    nc = _get_nc()
    in_maps = _prep_core_inputs(query, key, value, mask)
    core_ids = list(range(NCORES))
    res = run_bass_kernel_spmd(nc, in_maps, core_ids, trace=trace, **kw)

    out = np.empty((B, H, S, D), np.float32)
    p_attn = np.empty((B, H, S, S), np.float32)
    for c in range(NCORES):
        b, qi = c // (NCORES // B), c % (NCORES // B)
        q0 = qi * QBLK
        out[b, :, q0:q0 + QBLK, :] = res.results[c]["o_out"]
        p_attn[b, :, q0:q0 + QBLK, :] = res.results[c]["p_out"]
    return (out, p_attn), res


def kernel(query, key, value, mask):
    (out, p_attn), _ = run_on_cores(query, key, value, mask)
    return (out, p_attn)


# revision 20
# speedup vs baseline: 5.1648x; 5.1648x over previous
"""Trainium2 Bass kernel for nn_Attention_8589935148.

Attention with softmax over the HEAD axis (dim=1), returning (out, p_attn).

Math notes (B=2, H=16, S=2048, D=64):
  scores s[b,h,q,k] = (q . k) / 8;  mask[b,1,q,k] broadcasts over h, so a
  masked (b,q,k) position is masked for ALL 16 heads -> softmax over h of 16
  equal NEG values = uniform 1/16.  Exact reformulation used here:
      E   = exp(s)                      (raw, unmasked scores)
      Z   = (sum_h E - 16) * mask + 16  (= sum_h exp(s) unmasked; 16 masked)
      R   = 1/Z ; H = R*mask ; C = R - H
      p   = E*H + C                     (= softmax_h unmasked; 1/16 masked)
      out = p @ v

Everything on-chip lives in the TRANSPOSED orientation [k, q]:
  s^T[k,q] tiles come straight out of matmul(lhsT=K-tile, rhs=Q) with
  N=512-wide moving operand, the head-sum runs over [k,q] planes (PE
  accumulating identity matmuls), and the PV matmul consumes p^T directly
  as matmul(out^T[d,q], lhsT=V-tile[k,d], rhs=p^T[k,q]) -- no on-chip
  transposes at all. p_attn is written to HBM as bf16 [h, k, q] and the
  host un-transposes/upcasts; out is written as [h, d, q] f32, same deal.

Sharding: batch x query-block. Core c handles batch c//4, query rows
[(c%4)*512, (c%4+1)*512). All 16 heads stay on-core, so the softmax over
heads is purely local -- zero collectives.
"""

import sys

sys.path.insert(0, "/opt/trn_rl_repo")

import numpy as np
import ml_dtypes

import concourse.bass as bass
import concourse.tile as tile
from concourse import bacc, mybir
from concourse.bass_utils import run_bass_kernel_spmd
from concourse.masks import make_identity

F32 = mybir.dt.float32
F32R = mybir.dt.float32r
BF16 = mybir.dt.bfloat16
BF16_NP = np.dtype(ml_dtypes.bfloat16)

B, H, S, D = 2, 16, 2048, 64
NCORES = 8
QBLK = S // 4          # 512 query rows per core
KT = 128               # k tile (partition dim of the transposed world)
NKP = S // (2 * KT)    # 8 kt-pairs; each outer round covers 256 k rows
SCALE = 0.125          # 1/sqrt(64)

ALU = mybir.AluOpType
ACTF = mybir.ActivationFunctionType

# knobs
PE_ZSUM = True          # head-sum via PE accumulating identity matmuls
GP_TM_EVERY = 0         # >0: every Nth head's Tm multiply goes to GpSimd


def build_nc():
    nc = bacc.Bacc("TRN2", target_bir_lowering=False, debug=False,
                   num_devices=NCORES)

    # Inputs (per-core shards, pre-laid-out on host)
    qT = nc.dram_tensor("qT", [H // 2, 128, QBLK], F32R, kind="ExternalInput").ap()
    kT = nc.dram_tensor("kT", [H // 2, 128, S], F32R, kind="ExternalInput").ap()
    vp = nc.dram_tensor("vp", [S // KT, KT, H, D], BF16, kind="ExternalInput").ap()
    mkT = nc.dram_tensor("mkT", [NKP, 128, 2 * QBLK], BF16,
                         kind="ExternalInput").ap()
    # Outputs (transposed layouts; host fixes them up)
    p_out = nc.dram_tensor("p_out", [H, S, QBLK], BF16, kind="ExternalOutput").ap()
    o_out = nc.dram_tensor("o_out", [H, D, QBLK], F32, kind="ExternalOutput").ap()

    with tile.TileContext(nc) as tc:
        with (
            tc.tile_pool(name="const", bufs=1) as const_pool,
            tc.tile_pool(name="kTp", bufs=H // 2) as kT_pool,
            tc.tile_pool(name="qTp", bufs=H // 2) as qT_pool,
            tc.tile_pool(name="vpp", bufs=S // KT) as v_pool,
            tc.tile_pool(name="mkp", bufs=2) as m_pool,
            tc.tile_pool(name="Ep", bufs=17) as E_pool,
            tc.tile_pool(name="zp", bufs=2) as z_pool,
            tc.tile_pool(name="rp", bufs=2) as r_pool,
            tc.tile_pool(name="pbp", bufs=3) as pb_pool,
            tc.tile_pool(name="otp", bufs=8) as ot_pool,
            tc.tile_pool(name="spsum", bufs=2, space="PSUM") as s_psum,
            tc.tile_pool(name="zpsum", bufs=1, space="PSUM") as z_psum,
            tc.tile_pool(name="opsum", bufs=2, space="PSUM") as o_psum,
        ):
            ident = const_pool.tile([128, 128], BF16)
            make_identity(nc, ident[:])

            # ---- resident loads --------------------------------------
            kT_sb = []
            for i in range(H // 2):
                t = kT_pool.tile([128, S], F32R, tag="kT", name=f"kt{i}")
                nc.sync.dma_start(out=t, in_=kT[i])
                kT_sb.append(t)
            qT_sb = []
            for i in range(H // 2):
                t = qT_pool.tile([128, QBLK], F32R, tag="qT", name=f"qt{i}")
                nc.sync.dma_start(out=t, in_=qT[i])
                qT_sb.append(t)
            v_sb = []
            for i in range(S // KT):
                t = v_pool.tile([KT, H, D], BF16, tag="vp", name=f"v{i}")
                nc.sync.dma_start(out=t, in_=vp[i])
                v_sb.append(t)

            # out^T accumulators, 2 heads per tile (h -> parts 0:64/64:128)
            oT_sb = []
            for i in range(H // 2):
                t = ot_pool.tile([128, QBLK], F32, tag="oT", name=f"ot{i}")
                nc.vector.memset(t, 0.0)
                oT_sb.append(t)

            # ---- main loop: 8 rounds over kt-pairs -------------------
            for kp in range(NKP):
                mT = m_pool.tile([128, 2 * QBLK], BF16, tag="mkT",
                                 name=f"m{kp}")
                nc.sync.dma_start(out=mT, in_=mkT[kp])

                # scores + exp, all 16 heads
                Es = []
                for h in range(H):
                    pair, half = h // 2, h % 2
                    lo = 64 * half
                    s_ps = s_psum.tile([128, 2 * QBLK], F32, tag="sps",
                                       name=f"s{kp}_{h}")
                    for j in range(2):
                        ktile = kp * 2 + j
                        nc.tensor.matmul(
                            s_ps[:, j * QBLK:(j + 1) * QBLK],
                            lhsT=kT_sb[pair][lo:lo + 64,
                                             ktile * KT:(ktile + 1) * KT],
                            rhs=qT_sb[pair][lo:lo + 64, :],
                            start=True, stop=True,
                        )
                    E_h = E_pool.tile([128, 2 * QBLK], BF16, tag="E",
                                      name=f"E{kp}_{h}")
                    nc.scalar.activation(E_h, s_ps, ACTF.Exp, scale=SCALE)
                    Es.append(E_h)

                # Z = sum_h E via PE accumulating identity matmuls
                z_ps = z_psum.tile([128, 2 * QBLK], F32, tag="zps",
                                   name=f"z{kp}")
                for h in range(H):
                    for j in range(2):
                        nc.tensor.matmul(
                            z_ps[:, j * QBLK:(j + 1) * QBLK],
                            lhsT=ident,
                            rhs=Es[h][:, j * QBLK:(j + 1) * QBLK],
                            start=(h == 0), stop=(h == H - 1),
                        )

                # Zc = (Z-16)*mask + 16 ; R = 1/Zc ; H = R*mask ; C = R-H
                Zc = z_pool.tile([128, 2 * QBLK], F32, tag="Zc", name=f"zc{kp}")
                nc.vector.scalar_tensor_tensor(
                    out=Zc, in0=z_ps, scalar=-16.0, in1=mT,
                    op0=ALU.add, op1=ALU.mult)
                nc.vector.tensor_scalar_add(Zc, Zc, 16.0)
                R = r_pool.tile([128, 2 * QBLK], F32, tag="R", name=f"r{kp}")
                nc.vector.reciprocal_approx_fast(R, Zc)
                Hm = r_pool.tile([128, 2 * QBLK], BF16, tag="Hm", name=f"h{kp}")
                nc.vector.tensor_mul(Hm, R, mT)
                Cm = r_pool.tile([128, 2 * QBLK], BF16, tag="Cm", name=f"c{kp}")
                nc.vector.scalar_tensor_tensor(
                    out=Cm, in0=Hm, scalar=-1.0, in1=R,
                    op0=ALU.mult, op1=ALU.add)

                # normalize + PV per head (paired PSUM out accumulator)
                o_ps = None
                for h in range(H):
                    if h % 2 == 0:
                        o_ps = o_psum.tile([128, QBLK], F32, tag="ops",
                                           name=f"o{kp}_{h}")
                    Tm = pb_pool.tile([128, 2 * QBLK], BF16, tag="Tm",
                                      name=f"t{kp}_{h}")
                    teng = (nc.gpsimd if (GP_TM_EVERY and h % GP_TM_EVERY == 0)
                            else nc.vector)
                    teng.tensor_mul(Tm, Es[h], Hm)
                    Pb = pb_pool.tile([128, 2 * QBLK], BF16, tag="Pb",
                                      name=f"p{kp}_{h}")
                    nc.vector.tensor_add(Pb, Tm, Cm)
                    po = 64 * (h % 2)
                    for j in range(2):
                        ktile = kp * 2 + j
                        dma_eng = nc.sync if (h + j) % 2 == 0 else nc.scalar
                        dma_eng.dma_start(
                            out=p_out[h, ktile * KT:(ktile + 1) * KT, :],
                            in_=Pb[:, j * QBLK:(j + 1) * QBLK])
                        # out^T[d, q] += V[k,d].T @ p^T[k, q]
                        # each head opens/closes its own accumulation group
                        # over its own partition half of the shared bank
                        nc.tensor.matmul(
                            o_ps[po:po + 64, :],
                            lhsT=v_sb[ktile][:, h, :],
                            rhs=Pb[:, j * QBLK:(j + 1) * QBLK],
                            start=(j == 0), stop=(j == 1),
                        )
                    if h % 2 == 1:
                        acc = oT_sb[h // 2]
                        nc.vector.tensor_add(acc, acc, o_ps)

            # ---- write out^T ----------------------------------------
            for hp in range(H // 2):
                for half in range(2):
                    nc.sync.dma_start(
                        out=o_out[hp * 2 + half, :, :],
                        in_=oT_sb[hp][64 * half:64 * half + 64, :])

    nc.compile()
    return nc


_NC_CACHE = None


def _get_nc():
    global _NC_CACHE
    if _NC_CACHE is None:
        _NC_CACHE = build_nc()
    return _NC_CACHE


def _prep_core_inputs(query, key, value, mask):
    """Host-side shard + relayout. Returns list of 8 in_maps."""
    q = np.asarray(query, np.float32)
    k = np.asarray(key, np.float32)
    v = np.asarray(value, np.float32)
    m = np.asarray(mask)

    in_maps = []
    for b in range(B):
        # K^T packed: [16,2048,64] -> [16,64,2048] -> [8,128,2048]
        kTb = np.ascontiguousarray(k[b].transpose(0, 2, 1)).reshape(H // 2, 128, S)
        # V packed: [16,2048,64] -> [2048,16,64] -> [16,128,16,64] bf16
        vpb = np.ascontiguousarray(v[b].transpose(1, 0, 2)).reshape(
            S // KT, KT, H, D).astype(BF16_NP)
        for qi in range(NCORES // B):
            q0 = qi * QBLK
            qTb = np.ascontiguousarray(
                q[b, :, q0:q0 + QBLK, :].transpose(0, 2, 1)).reshape(
                H // 2, 128, QBLK)
            # mask^T [2048k, 512q] -> [8 kp, 128, 2 j, 512] -> [8, 128, 1024]
            mTb = np.ascontiguousarray(
                m[b, 0, q0:q0 + QBLK, :].T.astype(BF16_NP).reshape(
                    NKP, 2, 128, QBLK).transpose(0, 2, 1, 3).reshape(
                    NKP, 128, 2 * QBLK))
            in_maps.append({"qT": qTb, "kT": kTb, "vp": vpb, "mkT": mTb})
    return in_maps


def run_on_cores(query, key, value, mask, trace=False, **kw):
    nc = _get_nc()
    in_maps = _prep_core_inputs(query, key, value, mask)
    core_ids = list(range(NCORES))
    res = run_bass_kernel_spmd(nc, in_maps, core_ids, trace=trace, **kw)

    out = np.empty((B, H, S, D), np.float32)
    p_attn = np.empty((B, H, S, S), np.float32)
    for c in range(NCORES):
        b, qi = c // (NCORES // B), c % (NCORES // B)
        q0 = qi * QBLK
        # o_out [H, D, QBLK] -> [H, QBLK, D]
        out[b, :, q0:q0 + QBLK, :] = res.results[c]["o_out"].transpose(0, 2, 1)
        # p_out bf16 [H, S(k), QBLK(q)] -> f32 [H, QBLK, S]
        p_attn[b, :, q0:q0 + QBLK, :] = np.asarray(
            res.results[c]["p_out"], np.float32).transpose(0, 2, 1)
    return (out, p_attn), res


def kernel(query, key, value, mask):
    (out, p_attn), _ = run_on_cores(query, key, value, mask)
    return (out, p_attn)


# revision 23
# speedup vs baseline: 5.4585x; 1.0569x over previous
"""Trainium2 Bass kernel for nn_Attention_8589935148.

Attention with softmax over the HEAD axis (dim=1), returning (out, p_attn).

Math notes (B=2, H=16, S=2048, D=64):
  scores s[b,h,q,k] = (q . k) / 8;  mask[b,1,q,k] broadcasts over h, so a
  masked (b,q,k) position is masked for ALL 16 heads -> softmax over h of 16
  equal NEG values = uniform 1/16.  Exact reformulation used here:
      E   = exp(s)                      (raw, unmasked scores)
      Z   = (sum_h E - 16) * mask + 16  (= sum_h exp(s) unmasked; 16 masked)
      R   = 1/Z ; H = R*mask ; C = R - H
      p   = E*H + C                     (= softmax_h unmasked; 1/16 masked)
      out = p @ v

Everything on-chip lives in the TRANSPOSED orientation [k, q]:
  s^T[k,q] tiles come straight out of matmul(lhsT=K-tile, rhs=Q) with
  N=512-wide moving operand, the head-sum runs over [k,q] planes (PE
  accumulating identity matmuls), and the PV matmul consumes p^T directly
  as matmul(out^T[d,q], lhsT=V-tile[k,d], rhs=p^T[k,q]) -- no on-chip
  transposes at all. p_attn is written to HBM as bf16 [h, k, q] and the
  host un-transposes/upcasts; out is written as [h, d, q] f32, same deal.

Sharding: batch x query-block. Core c handles batch c//4, query rows
[(c%4)*512, (c%4+1)*512). All 16 heads stay on-core, so the softmax over
heads is purely local -- zero collectives.
"""

import sys

sys.path.insert(0, "/opt/trn_rl_repo")

import numpy as np
import ml_dtypes

import concourse.bass as bass
import concourse.tile as tile
from concourse import bacc, mybir
from concourse.bass_utils import run_bass_kernel_spmd
from concourse.masks import make_identity

F32 = mybir.dt.float32
F32R = mybir.dt.float32r
BF16 = mybir.dt.bfloat16
BF16_NP = np.dtype(ml_dtypes.bfloat16)

B, H, S, D = 2, 16, 2048, 64
NCORES = 8
QBLK = S // 4          # 512 query rows per core
KT = 128               # k tile (partition dim of the transposed world)
NKP = S // (2 * KT)    # 8 kt-pairs; each outer round covers 256 k rows
SCALE = 0.125          # 1/sqrt(64)

ALU = mybir.AluOpType
ACTF = mybir.ActivationFunctionType

# knobs
PE_ZSUM = True          # head-sum via PE accumulating identity matmuls
GP_TM_EVERY = 0         # >0: every Nth head's Tm multiply goes to GpSimd


def build_nc():
    nc = bacc.Bacc("TRN2", target_bir_lowering=False, debug=False,
                   num_devices=NCORES)

    # Inputs (per-core shards, pre-laid-out on host)
    qT = nc.dram_tensor("qT", [H // 2, 128, QBLK], F32R, kind="ExternalInput").ap()
    kT = nc.dram_tensor("kT", [H // 2, 128, S], F32R, kind="ExternalInput").ap()
    vp = nc.dram_tensor("vp", [S // KT, KT, H, D], BF16, kind="ExternalInput").ap()
    mkT = nc.dram_tensor("mkT", [NKP, 128, 2 * QBLK], BF16,
                         kind="ExternalInput").ap()
    # Outputs (transposed layouts; host fixes them up)
    p_out = nc.dram_tensor("p_out", [H, S, QBLK], BF16, kind="ExternalOutput").ap()
    o_out = nc.dram_tensor("o_out", [H, D, QBLK], F32, kind="ExternalOutput").ap()

    with tile.TileContext(nc) as tc:
        with (
            tc.tile_pool(name="const", bufs=1) as const_pool,
            tc.tile_pool(name="kTp", bufs=H // 2) as kT_pool,
            tc.tile_pool(name="qTp", bufs=H // 2) as qT_pool,
            tc.tile_pool(name="vpp", bufs=S // KT) as v_pool,
            tc.tile_pool(name="mkp", bufs=2) as m_pool,
            tc.tile_pool(name="Ep", bufs=17) as E_pool,
            tc.tile_pool(name="zp", bufs=2) as z_pool,
            tc.tile_pool(name="rp", bufs=2) as r_pool,
            tc.tile_pool(name="pbp", bufs=3) as pb_pool,
            tc.tile_pool(name="otp", bufs=8) as ot_pool,
            tc.tile_pool(name="spsum", bufs=2, space="PSUM") as s_psum,
            tc.tile_pool(name="zpsum", bufs=1, space="PSUM") as z_psum,
            tc.tile_pool(name="opsum", bufs=2, space="PSUM") as o_psum,
        ):
            ident = const_pool.tile([128, 128], BF16)
            make_identity(nc, ident[:])

            # ---- resident loads --------------------------------------
            kT_sb = []
            for i in range(H // 2):
                t = kT_pool.tile([128, S], F32R, tag="kT", name=f"kt{i}")
                nc.sync.dma_start(out=t, in_=kT[i])
                kT_sb.append(t)
            qT_sb = []
            for i in range(H // 2):
                t = qT_pool.tile([128, QBLK], F32R, tag="qT", name=f"qt{i}")
                nc.sync.dma_start(out=t, in_=qT[i])
                qT_sb.append(t)
            v_sb = []
            for i in range(S // KT):
                t = v_pool.tile([KT, H, D], BF16, tag="vp", name=f"v{i}")
                nc.sync.dma_start(out=t, in_=vp[i])
                v_sb.append(t)

            # out^T accumulators, 2 heads per tile (h -> parts 0:64/64:128)
            oT_sb = []
            for i in range(H // 2):
                t = ot_pool.tile([128, QBLK], F32, tag="oT", name=f"ot{i}")
                nc.vector.memset(t, 0.0)
                oT_sb.append(t)

            # ---- main loop: 8 rounds over kt-pairs -------------------
            for kp in range(NKP):
                mT = m_pool.tile([128, 2 * QBLK], BF16, tag="mkT",
                                 name=f"m{kp}")
                nc.sync.dma_start(out=mT, in_=mkT[kp])

                # scores + exp, all 16 heads
                Es = []
                for h in range(H):
                    pair, half = h // 2, h % 2
                    lo = 64 * half
                    s_ps = s_psum.tile([128, 2 * QBLK], F32, tag="sps",
                                       name=f"s{kp}_{h}")
                    for j in range(2):
                        ktile = kp * 2 + j
                        nc.tensor.matmul(
                            s_ps[:, j * QBLK:(j + 1) * QBLK],
                            lhsT=kT_sb[pair][lo:lo + 64,
                                             ktile * KT:(ktile + 1) * KT],
                            rhs=qT_sb[pair][lo:lo + 64, :],
                            start=True, stop=True,
                        )
                    E_h = E_pool.tile([128, 2 * QBLK], BF16, tag="E",
                                      name=f"E{kp}_{h}")
                    nc.scalar.activation(E_h, s_ps, ACTF.Exp, scale=SCALE)
                    Es.append(E_h)

                # Z = sum_h E via PE accumulating identity matmuls
                z_ps = z_psum.tile([128, 2 * QBLK], F32, tag="zps",
                                   name=f"z{kp}")
                for h in range(H):
                    for j in range(2):
                        nc.tensor.matmul(
                            z_ps[:, j * QBLK:(j + 1) * QBLK],
                            lhsT=ident,
                            rhs=Es[h][:, j * QBLK:(j + 1) * QBLK],
                            start=(h == 0), stop=(h == H - 1),
                        )

                # Zc = (Z-16)*mask + 16 ; R = 1/Zc ; H = R*mask ; C = R-H
                Zc = z_pool.tile([128, 2 * QBLK], F32, tag="Zc", name=f"zc{kp}")
                nc.vector.scalar_tensor_tensor(
                    out=Zc, in0=z_ps, scalar=-16.0, in1=mT,
                    op0=ALU.add, op1=ALU.mult)
                nc.vector.tensor_scalar_add(Zc, Zc, 16.0)
                R = r_pool.tile([128, 2 * QBLK], F32, tag="R", name=f"r{kp}")
                nc.vector.reciprocal_approx_fast(R, Zc)
                Hm = r_pool.tile([128, 2 * QBLK], BF16, tag="Hm", name=f"h{kp}")
                nc.vector.tensor_mul(Hm, R, mT)
                Cm = r_pool.tile([128, 2 * QBLK], BF16, tag="Cm", name=f"c{kp}")
                nc.vector.scalar_tensor_tensor(
                    out=Cm, in0=Hm, scalar=-1.0, in1=R,
                    op0=ALU.mult, op1=ALU.add)

                # normalize + PV per head (paired PSUM out accumulator)
                o_ps = None
                for h in range(H):
                    if h % 2 == 0:
                        o_ps = o_psum.tile([128, QBLK], F32, tag="ops",
                                           name=f"o{kp}_{h}")
                    Tm = pb_pool.tile([128, 2 * QBLK], BF16, tag="Tm",
                                      name=f"t{kp}_{h}")
                    teng = (nc.gpsimd if (GP_TM_EVERY and h % GP_TM_EVERY == 0)
                            else nc.vector)
                    teng.tensor_mul(Tm, Es[h], Hm)
                    Pb = pb_pool.tile([128, 2 * QBLK], BF16, tag="Pb",
                                      name=f"p{kp}_{h}")
                    nc.vector.tensor_add(Pb, Tm, Cm)
                    po = 64 * (h % 2)
                    for j in range(2):
                        ktile = kp * 2 + j
                        dma_eng = nc.sync if (h + j) % 2 == 0 else nc.scalar
                        dma_eng.dma_start(
                            out=p_out[h, ktile * KT:(ktile + 1) * KT, :],
                            in_=Pb[:, j * QBLK:(j + 1) * QBLK])
                        # out^T[d, q] += V[k,d].T @ p^T[k, q]
                        # each head opens/closes its own accumulation group
                        # over its own partition half of the shared bank
                        nc.tensor.matmul(
                            o_ps[po:po + 64, :],
                            lhsT=v_sb[ktile][:, h, :],
                            rhs=Pb[:, j * QBLK:(j + 1) * QBLK],
                            start=(j == 0), stop=(j == 1),
                        )
                    if h % 2 == 1:
                        acc = oT_sb[h // 2]
                        nc.vector.tensor_add(acc, acc, o_ps)

            # ---- write out^T ----------------------------------------
            for hp in range(H // 2):
                for half in range(2):
                    nc.sync.dma_start(
                        out=o_out[hp * 2 + half, :, :],
                        in_=oT_sb[hp][64 * half:64 * half + 64, :])

    nc.compile()
    return nc


_NC_CACHE = None


def _get_nc():
    global _NC_CACHE
    if _NC_CACHE is None:
        _NC_CACHE = build_nc()
    return _NC_CACHE


def _prep_core_inputs(query, key, value, mask):
    """Host-side shard + relayout. Returns list of 8 in_maps."""
    q = np.asarray(query, np.float32)
    k = np.asarray(key, np.float32)
    v = np.asarray(value, np.float32)
    m = np.asarray(mask)

    in_maps = []
    for b in range(B):
        # K^T packed: [16,2048,64] -> [16,64,2048] -> [8,128,2048]
        kTb = np.ascontiguousarray(k[b].transpose(0, 2, 1)).reshape(H // 2, 128, S)
        # V packed: [16,2048,64] -> [2048,16,64] -> [16,128,16,64] bf16
        vpb = np.ascontiguousarray(v[b].transpose(1, 0, 2)).reshape(
            S // KT, KT, H, D).astype(BF16_NP)
        for qi in range(NCORES // B):
            q0 = qi * QBLK
            qTb = np.ascontiguousarray(
                q[b, :, q0:q0 + QBLK, :].transpose(0, 2, 1)).reshape(
                H // 2, 128, QBLK)
            # mask^T [2048k, 512q] -> [8 kp, 128, 2 j, 512] -> [8, 128, 1024]
            mTb = np.ascontiguousarray(
                m[b, 0, q0:q0 + QBLK, :].T.astype(BF16_NP).reshape(
                    NKP, 2, 128, QBLK).transpose(0, 2, 1, 3).reshape(
                    NKP, 128, 2 * QBLK))
            in_maps.append({"qT": qTb, "kT": kTb, "vp": vpb, "mkT": mTb})
    return in_maps


def run_on_cores(query, key, value, mask, trace=False, **kw):
    nc = _get_nc()
    in_maps = _prep_core_inputs(query, key, value, mask)
    core_ids = list(range(NCORES))
    res = run_bass_kernel_spmd(nc, in_maps, core_ids, trace=trace, **kw)

    out = np.empty((B, H, S, D), np.float32)
    p_attn = np.empty((B, H, S, S), np.float32)
    for c in range(NCORES):
        b, qi = c // (NCORES // B), c % (NCORES // B)
        q0 = qi * QBLK
        # o_out [H, D, QBLK] -> [H, QBLK, D]
        out[b, :, q0:q0 + QBLK, :] = res.results[c]["o_out"].transpose(0, 2, 1)
        # p_out bf16 [H, S(k), QBLK(q)] -> f32 [H, QBLK, S]
        p_attn[b, :, q0:q0 + QBLK, :] = np.asarray(
            res.results[c]["p_out"], np.float32).transpose(0, 2, 1)
    return (out, p_attn), res


def kernel(query, key, value, mask):
    (out, p_attn), _ = run_on_cores(query, key, value, mask)
    return (out, p_attn)


# revision 24
# speedup vs baseline: 5.8942x; 1.0798x over previous
"""Trainium2 Bass kernel for nn_Attention_8589935148.

Attention with softmax over the HEAD axis (dim=1), returning (out, p_attn).

Math notes (B=2, H=16, S=2048, D=64):
  scores s[b,h,q,k] = (q . k) / 8;  mask[b,1,q,k] broadcasts over h, so a
  masked (b,q,k) position is masked for ALL 16 heads -> softmax over h of 16
  equal NEG values = uniform 1/16.  Exact reformulation used here:
      E   = exp(s)                      (raw, unmasked scores)
      Z   = (sum_h E - 16) * mask + 16  (= sum_h exp(s) unmasked; 16 masked)
      R   = 1/Z ; H = R*mask ; C = R - H
      p   = E*H + C                     (= softmax_h unmasked; 1/16 masked)
      out = p @ v

Everything on-chip lives in the TRANSPOSED orientation [k, q]:
  s^T[k,q] tiles come straight out of matmul(lhsT=K-tile, rhs=Q) with
  N=512-wide moving operand, the head-sum runs over [k,q] planes (PE
  accumulating identity matmuls), and the PV matmul consumes p^T directly
  as matmul(out^T[d,q], lhsT=V-tile[k,d], rhs=p^T[k,q]) -- no on-chip
  transposes at all. p_attn is written to HBM as bf16 [h, k, q] and the
  host un-transposes/upcasts; out is written as [h, d, q] f32, same deal.

Sharding: batch x query-block. Core c handles batch c//4, query rows
[(c%4)*512, (c%4+1)*512). All 16 heads stay on-core, so the softmax over
heads is purely local -- zero collectives.
"""

import sys

sys.path.insert(0, "/opt/trn_rl_repo")

import numpy as np
import ml_dtypes

import concourse.bass as bass
import concourse.tile as tile
from concourse import bacc, mybir
from concourse.bass_utils import run_bass_kernel_spmd
from concourse.masks import make_identity

F32 = mybir.dt.float32
F32R = mybir.dt.float32r
BF16 = mybir.dt.bfloat16
BF16_NP = np.dtype(ml_dtypes.bfloat16)

B, H, S, D = 2, 16, 2048, 64
NCORES = 8
QBLK = S // 4          # 512 query rows per core
KT = 128               # k tile (partition dim of the transposed world)
NKP = S // (2 * KT)    # 8 kt-pairs; each outer round covers 256 k rows
SCALE = 0.125          # 1/sqrt(64)

ALU = mybir.AluOpType
ACTF = mybir.ActivationFunctionType

# knobs
PE_ZSUM = True          # head-sum via PE accumulating identity matmuls
GP_TM_EVERY = 0         # >0: every Nth head's Tm multiply goes to GpSimd


def build_nc():
    nc = bacc.Bacc("TRN2", target_bir_lowering=False, debug=False,
                   num_devices=NCORES)

    # Inputs (per-core shards, pre-laid-out on host)
    qT = nc.dram_tensor("qT", [H // 2, 128, QBLK], BF16, kind="ExternalInput").ap()
    kT = nc.dram_tensor("kT", [H // 2, 128, S], BF16, kind="ExternalInput").ap()
    vp = nc.dram_tensor("vp", [S // KT, KT, H, D], BF16, kind="ExternalInput").ap()
    mkT = nc.dram_tensor("mkT", [NKP, 128, 2 * QBLK], BF16,
                         kind="ExternalInput").ap()
    # Outputs (transposed layouts; host fixes them up)
    p_out = nc.dram_tensor("p_out", [H, S, QBLK], BF16, kind="ExternalOutput").ap()
    o_out = nc.dram_tensor("o_out", [H, D, QBLK], F32, kind="ExternalOutput").ap()

    with tile.TileContext(nc) as tc:
        with (
            tc.tile_pool(name="const", bufs=1) as const_pool,
            tc.tile_pool(name="kTp", bufs=H // 2) as kT_pool,
            tc.tile_pool(name="qTp", bufs=H // 2) as qT_pool,
            tc.tile_pool(name="vpp", bufs=S // KT) as v_pool,
            tc.tile_pool(name="mkp", bufs=2) as m_pool,
            tc.tile_pool(name="Ep", bufs=9) as E_pool,
            tc.tile_pool(name="zp", bufs=2) as z_pool,
            tc.tile_pool(name="rp", bufs=2) as r_pool,
            tc.tile_pool(name="pbp", bufs=3) as pb_pool,
            tc.tile_pool(name="otp", bufs=8) as ot_pool,
            tc.tile_pool(name="spsum", bufs=2, space="PSUM") as s_psum,
            tc.tile_pool(name="zpsum", bufs=1, space="PSUM") as z_psum,
            tc.tile_pool(name="opsum", bufs=2, space="PSUM") as o_psum,
        ):
            ident = const_pool.tile([128, 128], BF16)
            make_identity(nc, ident[:])

            # ---- resident loads --------------------------------------
            kT_sb = []
            for i in range(H // 2):
                t = kT_pool.tile([128, S], BF16, tag="kT", name=f"kt{i}")
                nc.sync.dma_start(out=t, in_=kT[i])
                kT_sb.append(t)
            qT_sb = []
            for i in range(H // 2):
                t = qT_pool.tile([128, QBLK], BF16, tag="qT", name=f"qt{i}")
                nc.sync.dma_start(out=t, in_=qT[i])
                qT_sb.append(t)
            v_sb = []
            for i in range(S // KT):
                t = v_pool.tile([KT, H, D], BF16, tag="vp", name=f"v{i}")
                nc.sync.dma_start(out=t, in_=vp[i])
                v_sb.append(t)

            # out^T accumulators, 2 heads per tile (h -> parts 0:64/64:128)
            oT_sb = []
            for i in range(H // 2):
                t = ot_pool.tile([128, QBLK], F32, tag="oT", name=f"ot{i}")
                nc.vector.memset(t, 0.0)
                oT_sb.append(t)

            # ---- main loop: 8 rounds over kt-pairs -------------------
            for kp in range(NKP):
                mT = m_pool.tile([128, 2 * QBLK], BF16, tag="mkT",
                                 name=f"m{kp}")
                nc.sync.dma_start(out=mT, in_=mkT[kp])

                # scores + exp, all 16 heads; E pairs two heads per tile
                Etile = []
                for hp in range(H // 2):
                    Etile.append(E_pool.tile([128, 4 * QBLK], BF16, tag="E",
                                             name=f"E{kp}_{hp}"))
                Es = [Etile[h // 2][:, (h % 2) * 2 * QBLK:
                                    (h % 2 + 1) * 2 * QBLK] for h in range(H)]
                for h in range(H):
                    pair, half = h // 2, h % 2
                    lo = 64 * half
                    s_ps = s_psum.tile([128, 2 * QBLK], F32, tag="sps",
                                       name=f"s{kp}_{h}")
                    for j in range(2):
                        ktile = kp * 2 + j
                        nc.tensor.matmul(
                            s_ps[:, j * QBLK:(j + 1) * QBLK],
                            lhsT=kT_sb[pair][lo:lo + 64,
                                             ktile * KT:(ktile + 1) * KT],
                            rhs=qT_sb[pair][lo:lo + 64, :],
                            start=True, stop=True,
                        )
                    nc.scalar.activation(Es[h], s_ps, ACTF.Exp, scale=SCALE)

                # Z = sum_h E via PE accumulating identity matmuls
                z_ps = z_psum.tile([128, 2 * QBLK], F32, tag="zps",
                                   name=f"z{kp}")
                for h in range(H):
                    for j in range(2):
                        nc.tensor.matmul(
                            z_ps[:, j * QBLK:(j + 1) * QBLK],
                            lhsT=ident,
                            rhs=Es[h][:, j * QBLK:(j + 1) * QBLK],
                            start=(h == 0), stop=(h == H - 1),
                        )

                # Zc = (Z-16)*mask + 16 ; R = 1/Zc ; H = R*mask ; C = R-H
                Zc = z_pool.tile([128, 2 * QBLK], F32, tag="Zc", name=f"zc{kp}")
                nc.vector.scalar_tensor_tensor(
                    out=Zc, in0=z_ps, scalar=-16.0, in1=mT,
                    op0=ALU.add, op1=ALU.mult)
                nc.vector.tensor_scalar_add(Zc, Zc, 16.0)
                R = r_pool.tile([128, 2 * QBLK], F32, tag="R", name=f"r{kp}")
                nc.vector.reciprocal_approx_fast(R, Zc)
                Hm2 = r_pool.tile([128, 4 * QBLK], BF16, tag="Hm", name=f"h{kp}")
                Cm2 = r_pool.tile([128, 4 * QBLK], BF16, tag="Cm", name=f"c{kp}")
                for rep in range(2):
                    rsl = slice(rep * 2 * QBLK, (rep + 1) * 2 * QBLK)
                    nc.vector.tensor_mul(Hm2[:, rsl], R, mT)
                    nc.vector.scalar_tensor_tensor(
                        out=Cm2[:, rsl], in0=Hm2[:, rsl], scalar=-1.0, in1=R,
                        op0=ALU.mult, op1=ALU.add)

                # normalize (head-paired chain) + PV per head
                for hp in range(H // 2):
                    o_ps = o_psum.tile([128, QBLK], F32, tag="ops",
                                       name=f"o{kp}_{hp}")
                    Tm = pb_pool.tile([128, 4 * QBLK], BF16, tag="Tm",
                                      name=f"t{kp}_{hp}")
                    nc.vector.tensor_mul(Tm, Etile[hp], Hm2)
                    Pb = pb_pool.tile([128, 4 * QBLK], BF16, tag="Pb",
                                      name=f"p{kp}_{hp}")
                    nc.vector.tensor_add(Pb, Tm, Cm2)
                    for hh in range(2):
                        h = hp * 2 + hh
                        po = 64 * hh
                        for j in range(2):
                            ktile = kp * 2 + j
                            sl = slice((hh * 2 + j) * QBLK,
                                       (hh * 2 + j + 1) * QBLK)
                            dma_eng = nc.sync if (h + j) % 2 == 0 else nc.scalar
                            dma_eng.dma_start(
                                out=p_out[h, ktile * KT:(ktile + 1) * KT, :],
                                in_=Pb[:, sl])
                            nc.tensor.matmul(
                                o_ps[po:po + 64, :],
                                lhsT=v_sb[ktile][:, h, :],
                                rhs=Pb[:, sl],
                                start=(j == 0), stop=(j == 1),
                            )
                    acc = oT_sb[hp]
                    nc.vector.tensor_add(acc, acc, o_ps)

            # ---- write out^T ----------------------------------------
            for hp in range(H // 2):
                for half in range(2):
                    nc.sync.dma_start(
                        out=o_out[hp * 2 + half, :, :],
                        in_=oT_sb[hp][64 * half:64 * half + 64, :])

    nc.compile()
    return nc


_NC_CACHE = None


def _get_nc():
    global _NC_CACHE
    if _NC_CACHE is None:
        _NC_CACHE = build_nc()
    return _NC_CACHE


def _prep_core_inputs(query, key, value, mask):
    """Host-side shard + relayout. Returns list of 8 in_maps."""
    q = np.asarray(query, np.float32)
    k = np.asarray(key, np.float32)
    v = np.asarray(value, np.float32)
    m = np.asarray(mask)

    in_maps = []
    for b in range(B):
        # K^T packed: [16,2048,64] -> [16,64,2048] -> [8,128,2048]
        kTb = np.ascontiguousarray(k[b].transpose(0, 2, 1)).reshape(H // 2, 128, S).astype(BF16_NP)
        # V packed: [16,2048,64] -> [2048,16,64] -> [16,128,16,64] bf16
        vpb = np.ascontiguousarray(v[b].transpose(1, 0, 2)).reshape(
            S // KT, KT, H, D).astype(BF16_NP)
        for qi in range(NCORES // B):
            q0 = qi * QBLK
            qTb = np.ascontiguousarray(
                q[b, :, q0:q0 + QBLK, :].transpose(0, 2, 1)).reshape(
                H // 2, 128, QBLK).astype(BF16_NP)
            # mask^T [2048k, 512q] -> [8 kp, 128, 2 j, 512] -> [8, 128, 1024]
            mTb = np.ascontiguousarray(
                m[b, 0, q0:q0 + QBLK, :].T.astype(BF16_NP).reshape(
                    NKP, 2, 128, QBLK).transpose(0, 2, 1, 3).reshape(
                    NKP, 128, 2 * QBLK))
            in_maps.append({"qT": qTb, "kT": kTb, "vp": vpb, "mkT": mTb})
    return in_maps


def run_on_cores(query, key, value, mask, trace=False, **kw):
    nc = _get_nc()
    in_maps = _prep_core_inputs(query, key, value, mask)
    core_ids = list(range(NCORES))
    res = run_bass_kernel_spmd(nc, in_maps, core_ids, trace=trace, **kw)

    out = np.empty((B, H, S, D), np.float32)
    p_attn = np.empty((B, H, S, S), np.float32)
    for c in range(NCORES):
        b, qi = c // (NCORES // B), c % (NCORES // B)
        q0 = qi * QBLK
        # o_out [H, D, QBLK] -> [H, QBLK, D]
        out[b, :, q0:q0 + QBLK, :] = res.results[c]["o_out"].transpose(0, 2, 1)
        # p_out bf16 [H, S(k), QBLK(q)] -> f32 [H, QBLK, S]
        p_attn[b, :, q0:q0 + QBLK, :] = np.asarray(
            res.results[c]["p_out"], np.float32).transpose(0, 2, 1)
    return (out, p_attn), res


def kernel(query, key, value, mask):
    (out, p_attn), _ = run_on_cores(query, key, value, mask)
    return (out, p_attn)
